# revision 76
# baseline (speedup 1.0000x reference)
"""Trainium2 Bass kernel for nn_AttentionHead_48077863911491.

Computation (per batch b of 4):
    q = h @ Wq + bq            [S=2048, D=64]
    k = h @ Wk + bk            [S, D]
    scores = (q @ k^T) / 8     [Sq, Sk]
    w = softmax(scores, axis=0)   # over the QUERY axis (per key column)
    out = w @ h                [Sq, E=1024]   # h (not v) is the value tensor

Sharding: 8 cores = 4 batches x 2 key-halves (identical SPMD program; the
half=1 core's queries are rolled by -1024 so its keys are rows 0:1024; the
host sums the two key-half partials and rolls back).

Per core: A) combined QK projection (one [128,128] = [Wq'|Wk] stationary
block per e-tile); B) per key-tile kt: transposed scores (keys on
partitions), exp with free-axis accumulation -> per-key 1/sum folded into
the value rows hs = hk * rinv * m; C) outT accumulated per e-tile.

Precision plan (rel-L2 gate 2e-2; numpy model agrees with HW to ~1e-7):
  - corrected pair-halves: keys 0:512 (kt0-3) use fp8 DoubleRow with both
    residual terms: H*W + h*W + H*v where X = fp8(x), x_res = fp8(x - X).
  - raw half: keys 512:1024 (kt4-7) use a single H*W DoubleRow.
  - if WAVE: e-tile 0 runs fully in fp16 (error 1.80e-2); else all 32
    out-tiles are fp8 (error 1.92e-2).
  - if RAW_DIRECT: the raw half's exp and hk*rinv write fp8 operands
    directly (no f16 intermediates / conversion copies).

Schedule (65612 ns on the instruction-cost timeline, vs 70713 baseline):
  - hT streams in q-column HALVES on the serial DMA device, so the P1
    (q 0:1024 Q|K) projection finishes after half the input stream and
    the ACT exp pipeline (the phase-B wall: 16 exps x 1225 ns incl. the
    187 ns accumulator read) starts ~4us earlier; the P2 (q 1024:2048 Q)
    projection fills PE in the first exp sweep's shadow, emitted one
    e-tile ahead so its stop-matmul clears before the h0 scores finish.
  - a dummy activation at t~0 absorbs the 1.3us ACT table load; K bias
    runs on ACT and Q bias on DVE (only two readers of the P1 PSUM -
    same-tile readers serialize ~220ns apart).
  - corrected-pair conversions run inside phase B on DVE/GPSIMD (h0
    parts during the h0 sweep; DVE copies deferred one kt so the
    rinv->hs chain stays prompt for the wave); raw-half conversions run
    right after B on the then-idle ACT plus DVE/GPSIMD.
  - PSUM: A uses split P1/P2 pools; each exp sweep has a 2-deep sc ring;
    the fp16 wave holds 4 banks from the h1 sweep on; phase C double-
    buffers groups of 4 out-tiles across two pools, emitting the raw DRs
    of group g after the corrected DRs of group g+1 so the in-order PE
    does not wait on late raw conversions.
  - the fp16 wave's matmuls are priority-sorted BELOW the h1 sweep
    (tc.high_priority(offset=-400)) so the compile-time list scheduler
    never slots one where it delays the next scores -> exp handoff;
    this keeps the exp stream continuous and phase C starts the moment
    the last accumulator read retires.
  - outputs drain as half-tile DMAs per group, with the last e-tile
    split into 2-/1-tile groups ending in quarter DMAs; the penultimate
    quarter issues via the Pool SWDGE path so the final quarter gets
    the HWDGE queue without waiting, keeping the ~2.5us last-DMA chain
    off the critical tail.
"""

import numpy as np

import concourse.bass as bass
import concourse.mybir as mybir
import concourse.tile as tile
from concourse import bacc
from concourse.bass_utils import run_bass_kernel_spmd

B, S, E, D = 4, 2048, 1024, 64
KH = S // 2          # keys per core
P = 128
ET = E // P          # 8 e-tiles
KT = KH // P         # 8 key-tiles per core
NP = KT // 2         # 4 kt-pairs
CORR = (0, 1)        # corrected pairs (keys 0:512)
RAW = (2, 3)         # raw pairs (keys 512:1024)
SCALE = 1.0 / np.sqrt(D)
C_SHIFT = 2.5        # exp(s - C): max exp ~134 < 240 (fp8e4-safe range)
M_SCALE = 2.0 ** 13  # hs scaling into fp8e4 normal range

WAVE = True          # e-tile 0 in fp16
RAW_DIRECT = False   # raw half exp/hs write fp8 directly

_cached = {}


def build_bass(wave=WAVE, raw_direct=RAW_DIRECT):
    f16, f32 = mybir.dt.float16, mybir.dt.float32
    e4 = mybir.dt.float8e4
    DR = mybir.MatmulPerfMode.DoubleRow
    EXP = mybir.ActivationFunctionType.Exp
    nc = bacc.Bacc("TRN2", target_bir_lowering=False, debug=False, num_devices=8)

    hT = nc.dram_tensor("hT", [E, S], f16, kind="ExternalInput").ap()
    hk = nc.dram_tensor("hk", [KH, E], f16, kind="ExternalInput").ap()
    wqk = nc.dram_tensor("wqk", [P, ET * P], f16, kind="ExternalInput").ap()
    bqk = nc.dram_tensor("bqk", [P, 1], f32, kind="ExternalInput").ap()
    outT = nc.dram_tensor("outT", [E, S], f16, kind="ExternalOutput").ap()

    hT3 = hT.rearrange("(t p) q -> t p q", p=P)      # [8, 128, 2048]
    hk3 = hk.rearrange("(t p) e -> t p e", p=P)      # [8, 128, 1024]
    outT3 = outT.rearrange("(t p) q -> t p q", p=P)  # [8, 128, 2048]

    with tile.TileContext(nc) as tc:
        with (
            tc.tile_pool(name="p_in", bufs=ET) as p_in,
            tc.tile_pool(name="p_w", bufs=1) as p_w,
            tc.tile_pool(name="p_soft", bufs=KT) as p_soft,
            tc.tile_pool(name="p_f8", bufs=NP) as p_f8,
            tc.tile_pool(name="p_out", bufs=ET) as p_out,
        ):
            # ---- input DMAs ----
            # hT streams in q-column halves: sweep 1 (q 0:1024) feeds the
            # P1 projection so the exp pipeline starts after HALF the input
            # stream; sweep 2 (q 1024:2048) feeds P2 in the exp shadow.
            wqk_sb = p_w.tile([P, ET, P], f16, tag="wqk")
            nc.sync.dma_start(wqk_sb[:, 0, :], wqk[:, 0:P])
            hTa = [p_in.tile([P, S // 2], f16, tag="hTa", name=f"hTa_{et}")
                   for et in range(ET)]
            hTb = [p_in.tile([P, S // 2], f16, tag="hTb", name=f"hTb_{et}")
                   for et in range(ET)]
            nc.sync.dma_start(hTa[0][:], hT3[0][:, 0:1024])
            nc.sync.dma_start(
                wqk_sb[:, 1:, :].rearrange("p t c -> p (t c)"),
                wqk[:, P:ET * P])
            for et in range(1, ET):
                nc.sync.dma_start(hTa[et][:], hT3[et][:, 0:1024])
            bqk_sb = p_w.tile([P, 1], f32, tag="bqk")
            nc.sync.dma_start(bqk_sb[:], bqk[:])
            for et in range(ET):
                nc.sync.dma_start(hTb[et][:], hT3[et][:, 1024:2048])
            hk_sb = []
            for kt in range(KT):
                t = p_in.tile([P, E], f16, tag="hk", name=f"hk_{kt}")
                nc.sync.dma_start(t[:], hk3[kt])
                hk_sb.append(t)

            QT16h = [p_w.tile([D, S // 2], f16, tag=f"qt{h}", name=f"QT16_{h}")
                     for h in range(2)]
            KT16 = p_w.tile([D, KH], f16, tag="kt16")
            cshift = p_w.tile([P, 1], f32, tag="cshift")
            nc.gpsimd.memset(cshift[:], -C_SHIFT)
            # dummy activation: pulls the 1.3us activation-table load into
            # the DMA-bound startup window instead of the exp critical path
            warm = p_w.tile([P, 1], f32, tag="warm")
            nc.scalar.activation(warm[:], cshift[:], EXP)

            # persistent operand tiles
            w16 = []      # per kt: [128, 2048] f16
            hs16 = []     # per kt: [128, 1024] f16
            ssum_a = p_w.tile([P, KT], f32, tag="ssum_a")
            ssum_b = p_w.tile([P, KT], f32, tag="ssum_b")
            rinv = p_w.tile([P, KT], f32, tag="rinv")
            Wp = [p_f8.tile([P, 2, S], e4, tag="W8", name=f"W8_{p}")
                  for p in range(NP)]
            vp = {p: p_f8.tile([P, 2, S], e4, tag="v8", name=f"v8_{p}")
                  for p in CORR}
            Hp = [p_f8.tile([P, 2, E], e4, tag="H8", name=f"H8_{p}")
                  for p in range(NP)]
            hp = {p: p_f8.tile([P, 2, E], e4, tag="h8", name=f"h8_{p}")
                  for p in CORR}

            ot_et = {}

            def evict(et, i, eng):
                if et not in ot_et:
                    ot_et[et] = p_out.tile([P, S], f16, tag="ot",
                                           name=f"ot_{et}")
                eng(ot_et[et][:, i * 512:(i + 1) * 512], psc[(et, i)][:])

            def dma_q(et, i):
                nc.sync.dma_start(
                    outT3[et][:, i * 512:(i + 1) * 512],
                    ot_et[et][:, i * 512:(i + 1) * 512])

            psc = {}

            # ---- phase A (P1) + sweep h0 ----
            with tc.tile_pool(name="ps_p2", bufs=1, space="PSUM") as ps_p2:
                P2 = ps_p2.tile([D, S // 2], f32, tag="p2")
                with tc.tile_pool(name="ps_p1", bufs=1,
                                  space="PSUM") as ps_p1:
                    P1 = ps_p1.tile([P, S // 2], f32, tag="p1")
                    for et in range(ET):
                        st, sp = (et == 0), (et == ET - 1)
                        for c in range(2):
                            nc.tensor.matmul(
                                P1[:, c * 512:(c + 1) * 512],
                                wqk_sb[:, et, :],
                                hTa[et][:, c * 512:(c + 1) * 512],
                                start=st, stop=sp)
                    # KT bias halves on ACT (idle until the exps), QT0 bias
                    # halves on DVE, in parallel, so kt0's scores start fast
                    # two P1 readers only (same-tile readers serialize
                    # ~220ns apart): Q bias on ACT (1038 < DVE's 1192),
                    # K bias on DVE
                    with tc.high_priority():
                        nc.scalar.activation(
                            QT16h[0][:], P1[0:D, :],
                            mybir.ActivationFunctionType.Identity,
                            bias=bqk_sb[0:D, 0:1])
                        nc.vector.tensor_scalar_add(
                            KT16[:], P1[D:P, :], bqk_sb[D:P, 0:1])

                with tc.tile_pool(name="ps_b0", bufs=3,
                                  space="PSUM") as ps_b0:
                    # -- sweep h0: scores+exp on q 0:1024; P2 fills PE --
                    for kt in range(KT):
                        pi, j = divmod(kt, 2)
                        wt = p_soft.tile([P, S], f16, tag="w16",
                                         name=f"w16_{kt}")
                        w16.append(wt)
                        sc = ps_b0.tile([P, S // 2], f32, tag="sc",
                                        name=f"sc_{kt}_0")
                        for c in range(2):
                            nc.tensor.matmul(
                                sc[:, c * 512:(c + 1) * 512],
                                KT16[:, kt * P:(kt + 1) * P],
                                QT16h[0][:, c * 512:(c + 1) * 512],
                                start=True, stop=True)
                        nc.scalar.activation(
                            wt[:, 0:1024], sc[:], EXP, bias=cshift[:],
                            accum_out=ssum_a[:, kt:kt + 1])
                        # P2 projection consumes sweep 2 as it lands, one
                        # e-tile ahead so its stop-matmul (and the QT1 bias
                        # behind it) clears before the h0 scores finish
                        p2_ets = ([0, 1] if kt == 0 else
                                  [kt + 1] if kt < ET - 1 else [])
                        with tc.high_priority(offset=-400):
                            for et in p2_ets:
                                for c in range(2):
                                    nc.tensor.matmul(
                                        P2[:, c * 512:(c + 1) * 512],
                                        wqk_sb[:, et, 0:D],
                                        hTb[et][:, c * 512:(c + 1) * 512],
                                        start=(et == 0), stop=(et == ET - 1))
                        if pi in CORR and kt < 2:
                            # corrected h0 conversions (DVE/Pool idle now),
                            # sorted below the sweep so they never delay the
                            # scores/exp/bias stream; kt2/3's wait until
                            # after the QT1 bias
                            with tc.high_priority(offset=-400):
                                nc.vector.tensor_copy(
                                    Wp[pi][:, j, 0:1024], wt[:, 0:1024])
                                nc.gpsimd.tensor_sub(
                                    vp[pi][:, j, 0:1024], wt[:, 0:1024],
                                    Wp[pi][:, j, 0:1024])
                    nc.vector.tensor_scalar_add(
                        QT16h[1][:], P2[0:D, :], bqk_sb[0:D, 0:1])
                    for kt in (2, 3):
                        pi, j = divmod(kt, 2)
                        nc.vector.tensor_copy(
                            Wp[pi][:, j, 0:1024], w16[kt][:, 0:1024])
                        nc.gpsimd.tensor_sub(
                            vp[pi][:, j, 0:1024], w16[kt][:, 0:1024],
                            Wp[pi][:, j, 0:1024])

            # ---- sweep h1 + phase C ----
            with tc.tile_pool(name="ps_w", bufs=4, space="PSUM") as ps_w:
                wave_psc = None
                if wave:
                    wave_psc = [ps_w.tile([P, 512], f32, tag="ops",
                                          name=f"wv_{i}") for i in range(4)]
                    for i in range(4):
                        psc[(0, i)] = wave_psc[i]

                def wave_adv(kt):
                    for i in range(4):
                        nc.tensor.matmul(
                            wave_psc[i][:], hs16[kt][:, 0:P],
                            w16[kt][:, i * 512:(i + 1) * 512],
                            start=(kt == 0), stop=(kt == KT - 1))

                conv_q = []
                with tc.tile_pool(name="ps_b1", bufs=2,
                                  space="PSUM") as ps_b1:
                    # -- sweep h1: scores+exp on q 1024:2048 + chains --
                    for kt in range(KT):
                        pi, j = divmod(kt, 2)
                        wt = w16[kt]
                        sc = ps_b1.tile([P, S // 2], f32, tag="sc",
                                        name=f"sc_{kt}_1")
                        for c in range(2):
                            nc.tensor.matmul(
                                sc[:, c * 512:(c + 1) * 512],
                                KT16[:, kt * P:(kt + 1) * P],
                                QT16h[1][:, c * 512:(c + 1) * 512],
                                start=True, stop=True)
                        nc.scalar.activation(
                            wt[:, 1024:2048], sc[:], EXP, bias=cshift[:],
                            accum_out=ssum_b[:, kt:kt + 1])
                        with tc.high_priority():
                            # ssum->rinv->hs gates the wave and H8
                            nc.vector.tensor_add(
                                rinv[:, kt:kt + 1],
                                ssum_a[:, kt:kt + 1], ssum_b[:, kt:kt + 1])
                            nc.vector.reciprocal_approx_fast(
                                rinv[:, kt:kt + 1], rinv[:, kt:kt + 1])
                            nc.vector.tensor_scalar_mul(
                                rinv[:, kt:kt + 1], rinv[:, kt:kt + 1],
                                M_SCALE)
                            hs = p_soft.tile([P, E], f16, tag="hs",
                                             name=f"hs_{kt}")
                            nc.vector.tensor_scalar_mul(
                                hs[:], hk_sb[kt][:], rinv[:, kt:kt + 1])
                        hs16.append(hs)
                        if pi in CORR:
                            # corrected h1 conversions; DVE copies deferred
                            # one kt so the next rinv->hs chain isn't stuck
                            # behind them on the in-order DVE; residual subs
                            # split across Pool/DVE by parity
                            def conv_h1(kt=kt, pi=pi, j=j, wt=wt, hs=hs):
                                nc.vector.tensor_copy(
                                    Wp[pi][:, j, 1024:2048], wt[:, 1024:2048])
                                nc.vector.tensor_copy(Hp[pi][:, j, :], hs[:])
                                if kt % 2 == 0:
                                    nc.gpsimd.tensor_sub(
                                        vp[pi][:, j, 1024:2048],
                                        wt[:, 1024:2048],
                                        Wp[pi][:, j, 1024:2048])
                                    nc.gpsimd.tensor_sub(
                                        hp[pi][:, j, :], hs[:],
                                        Hp[pi][:, j, :])
                                else:
                                    nc.vector.tensor_sub(
                                        vp[pi][:, j, 1024:2048],
                                        wt[:, 1024:2048],
                                        Wp[pi][:, j, 1024:2048])
                                    nc.vector.tensor_sub(
                                        hp[pi][:, j, :], hs[:],
                                        Hp[pi][:, j, :])
                            conv_q.append(conv_h1)
                        if conv_q and kt >= 1:
                            conv_q.pop(0)()
                        if wave and kt >= 2:
                            # sort the wave BELOW the sweep's scores/exps so
                            # the compile-time scheduler never slots a wave
                            # matmul where it delays the next scores -> exp
                            with tc.high_priority(offset=-400):
                                wave_adv(kt - 2)

                for fn in conv_q:
                    fn()
                conv_q.clear()

                # ---- phase C ----
                with tc.tile_pool(name="ps_c", bufs=4, space="PSUM") as ps_c:
                    if wave:
                        # finish + evict the wave FIRST (all on ACT, ahead of
                        # its raw-conversion queue): frees the wave's PSUM
                        # ring for group 2's corrected DRs
                        wave_adv(KT - 2)
                        wave_adv(KT - 1)
                        with tc.high_priority():
                            for i in range(4):
                                evict(0, i, nc.scalar.copy)
                                dma_q(0, i)
                        groups = ([(e, (0, 1, 2, 3))
                                   for e in range(1, ET - 2)]
                                  + [(ET - 2, (0, 1)), (ET - 2, (2, 3)),
                                     (ET - 1, (0, 1)), (ET - 1, (2,)),
                                     (ET - 1, (3,))])
                    else:
                        groups = ([(e, (0, 1, 2, 3)) for e in range(ET - 1)]
                                  + [(ET - 1, (0, 1)), (ET - 1, (2, 3))])

                    # raw-half conversions (ACT idle post-exps; DVE/Pool
                    # pick up after their corrected-pair backlog)
                    engs = {
                        4: (nc.scalar.copy, nc.scalar.copy),
                        5: (nc.gpsimd.tensor_copy, nc.gpsimd.tensor_copy),
                        6: (nc.scalar.copy, nc.gpsimd.tensor_copy),
                        7: (nc.vector.tensor_copy, nc.vector.tensor_copy),
                    }
                    for kt in (4, 5, 6, 7):
                        pi, j = divmod(kt, 2)
                        weng, heng = engs[kt]
                        weng(Wp[pi][:, j, :], w16[kt][:])
                        heng(Hp[pi][:, j, :], hs16[kt][:])
                    pools = [ps_c, ps_w]

                    def group_corr(g, pairs=CORR):
                        et, ilist = groups[g]
                        if pairs[0] == CORR[0]:
                            pool = pools[g % 2]
                            for i in ilist:
                                psc[(et, i)] = pool.tile(
                                    [P, 512], f32, tag="ops",
                                    name=f"psc_{et}_{i}")
                        for p in pairs:
                            for i in ilist:
                                es = slice(et * P, (et + 1) * P)
                                qs = slice(i * 512, (i + 1) * 512)
                                nc.tensor.matmul(
                                    psc[(et, i)][:], Hp[p][:, :, es],
                                    Wp[p][:, :, qs],
                                    start=(p == CORR[0]), stop=False,
                                    perf_mode=DR)
                                nc.tensor.matmul(
                                    psc[(et, i)][:], hp[p][:, :, es],
                                    Wp[p][:, :, qs],
                                    start=False, stop=False, perf_mode=DR)
                                nc.tensor.matmul(
                                    psc[(et, i)][:], Hp[p][:, :, es],
                                    vp[p][:, :, qs],
                                    start=False, stop=False, perf_mode=DR)

                    def group_raw_finish(g, last):
                        # raw DRs, then evict right behind each tile's final
                        # DR. The last two groups go per-tile-major with
                        # quarter DMAs so the output stream drains during
                        # the remaining DRs instead of after them.
                        et, ilist = groups[g]
                        es = slice(et * P, (et + 1) * P)
                        for i in ilist:
                            qs = slice(i * 512, (i + 1) * 512)
                            nc.tensor.matmul(
                                psc[(et, i)][:], Hp[RAW[0]][:, :, es],
                                Wp[RAW[0]][:, :, qs],
                                start=False, stop=False, perf_mode=DR)
        # early groups' evicts/DMAs have slack until their PSUM
                        # ring slot is re-needed; sort them below the raw
                        # conversions competing for DVE/ACT at C-start
                        early = g < 3
                        for i in ilist:
                            qs = slice(i * 512, (i + 1) * 512)
                            nc.tensor.matmul(
                                psc[(et, i)][:], Hp[RAW[1]][:, :, es],
                                Wp[RAW[1]][:, :, qs],
                                start=False, stop=True, perf_mode=DR)
                            with tc.high_priority(offset=-150 if early else None):
                                evict(et, i, (nc.vector.tensor_copy
                                              if i % 2 == 0
                                              else nc.scalar.copy))
                            if g == len(groups) - 2 and i >= 2:
                                # penultimate tile's quarter via the Pool
                                # SWDGE path: no HWDGE slot, so the final
                                # quarter issues without queueing
                                nc.gpsimd.dma_start(
                                    outT3[et][:, i * 512:(i + 1) * 512],
                                    ot_et[et][:, i * 512:(i + 1) * 512])
                            elif g == len(groups) - 1 and i >= 2:
                                dma_q(et, i)
                            elif i == 1 or i == 3:
                                hs_ = slice((i // 2) * 1024,
                                            (i // 2) * 1024 + 1024)
                                if g >= len(groups) - 5:
                                    # late halves via SWDGE (Pool idle by
                                    # then): keeps HWDGE clear for the drain
                                    nc.gpsimd.dma_start(outT3[et][:, hs_],
                                                        ot_et[et][:, hs_])
                                else:
                                    nc.sync.dma_start(outT3[et][:, hs_],
                                                      ot_et[et][:, hs_])

                    ng = len(groups)
                    # first two groups pair-interleaved: G1-p0 fills the PE
                    # while kt2/3's conversions finish for the p1 DRs
                    group_corr(0, (CORR[0],))
                    group_corr(1, (CORR[0],))
                    group_corr(0, (CORR[1],))
                    group_corr(1, (CORR[1],))
                    for g in range(ng):
                        if 1 <= g and g + 1 < ng:
                            group_corr(g + 1)
                        group_raw_finish(g, last=(g == ng - 1))

    nc.compile()
    return nc


def _prep_in_maps(h, Wq, bq, Wk, bk):
    wq16 = (np.asarray(Wq, np.float32) * SCALE).astype(np.float16)
    wk16 = np.asarray(Wk, np.float32).astype(np.float16)
    # per e-tile stationary [128, 128] = [Wq' | Wk] rows for that tile
    wqk = np.concatenate(
        [wq16.reshape(ET, P, D), wk16.reshape(ET, P, D)], axis=2)  # [ET,128,128]
    wqk = np.ascontiguousarray(wqk.transpose(1, 0, 2).reshape(P, ET * P))
    bqk = np.ascontiguousarray(np.concatenate(
        [np.asarray(bq, np.float32) * SCALE,
         np.asarray(bk, np.float32)]).reshape(P, 1))
    in_maps = []
    for c in range(8):
        b, half = divmod(c, 2)
        hb = np.asarray(h[b], np.float32)
        rolled = np.roll(hb, -KH * half, axis=0) if half else hb
        h16 = rolled.astype(np.float16)
        in_maps.append({
            "hT": np.ascontiguousarray(h16.T),
            "hk": np.ascontiguousarray(h16[0:KH]),
            "wqk": wqk, "bqk": bqk,
        })
    return in_maps


def _assemble(results):
    out = np.empty((B, S, E), np.float32)
    inv_m = np.float32(1.0 / M_SCALE)
    for b in range(B):
        p0 = results[2 * b]["outT"].astype(np.float32).T
        p1 = results[2 * b + 1]["outT"].astype(np.float32).T
        out[b] = (p0 + np.roll(p1, KH, axis=0)) * inv_m
    return out


def kernel(h, Wq, bq, Wk, bk, Wv=None, bv=None, **_unused):
    if "nc" not in _cached:
        _cached["nc"] = build_bass()
    nc = _cached["nc"]
    in_maps = _prep_in_maps(h, Wq, bq, Wk, bk)
    res = run_bass_kernel_spmd(nc, in_maps, list(range(8)))
    return _assemble(res.results)


# revision 77
# speedup vs baseline: 1.0039x; 1.0039x over previous
"""Trainium2 Bass kernel for nn_AttentionHead_48077863911491.

Computation (per batch b of 4):
    q = h @ Wq + bq            [S=2048, D=64]
    k = h @ Wk + bk            [S, D]
    scores = (q @ k^T) / 8     [Sq, Sk]
    w = softmax(scores, axis=0)   # over the QUERY axis (per key column)
    out = w @ h                [Sq, E=1024]   # h (not v) is the value tensor

Sharding: 8 cores = 4 batches x 2 key-halves (identical SPMD program; the
half=1 core's queries are rolled by -1024 so its keys are rows 0:1024; the
host sums the two key-half partials and rolls back).

Per core: A) combined QK projection (one [128,128] = [Wq'|Wk] stationary
block per e-tile); B) per key-tile kt: transposed scores (keys on
partitions), exp with free-axis accumulation -> per-key 1/sum folded into
the value rows hs = hk * rinv * m; C) outT accumulated per e-tile.

Precision plan (rel-L2 gate 2e-2; numpy model agrees with HW to ~1e-7):
  - corrected pair-halves: keys 0:512 (kt0-3) use fp8 DoubleRow with both
    residual terms: H*W + h*W + H*v where X = fp8(x), x_res = fp8(x - X).
  - raw half: keys 512:1024 (kt4-7) use a single H*W DoubleRow.
  - if WAVE: e-tile 0 runs fully in fp16 (error 1.80e-2); else all 32
    out-tiles are fp8 (error 1.92e-2).
  - if RAW_DIRECT: the raw half's exp and hk*rinv write fp8 operands
    directly (no f16 intermediates / conversion copies).

Schedule (65600 ns on the instruction-cost timeline, vs 70713 baseline):
  - hT streams in q-column HALVES on the serial DMA device, so the P1
    (q 0:1024 Q|K) projection finishes after half the input stream and
    the ACT exp pipeline (the phase-B wall: 16 exps x 1225 ns incl. the
    187 ns accumulator read) starts ~4us earlier; the P2 (q 1024:2048 Q)
    projection fills PE in the first exp sweep's shadow, emitted one
    e-tile ahead so its stop-matmul clears before the h0 scores finish.
  - a dummy activation at t~0 absorbs the 1.3us ACT table load; Q bias
    runs on ACT and K bias on DVE (only two readers of the P1 PSUM -
    same-tile readers serialize ~220ns apart).
  - corrected-pair conversions run inside phase B on DVE/GPSIMD (h0
    parts during the h0 sweep; DVE copies deferred one kt so the
    rinv->hs chain stays prompt for the wave); raw-half conversions run
    right after B on the then-idle ACT plus DVE/GPSIMD.
  - PSUM: A uses split P1/P2 pools; each exp sweep has a 2-deep sc ring;
    the fp16 wave holds 4 banks from the h1 sweep on; phase C double-
    buffers groups of 4 out-tiles across two pools, emitting the raw DRs
    of group g after the corrected DRs of group g+1 so the in-order PE
    does not wait on late raw conversions.
  - the fp16 wave's matmuls are priority-sorted BELOW the h1 sweep
    (tc.high_priority(offset=-400)) so the compile-time list scheduler
    never slots one where it delays the next scores -> exp handoff;
    this keeps the exp stream continuous and phase C starts the moment
    the last accumulator read retires.
  - outputs drain as half-tile DMAs per group, with the last e-tile
    split into 2-/1-tile groups ending in quarter DMAs; the penultimate
    quarter issues via the Pool SWDGE path so the final quarter gets
    the HWDGE queue without waiting, keeping the ~2.5us last-DMA chain
    off the critical tail.
"""

import numpy as np

import concourse.bass as bass
import concourse.mybir as mybir
import concourse.tile as tile
from concourse import bacc
from concourse.bass_utils import run_bass_kernel_spmd

B, S, E, D = 4, 2048, 1024, 64
KH = S // 2          # keys per core
P = 128
ET = E // P          # 8 e-tiles
KT = KH // P         # 8 key-tiles per core
NP = KT // 2         # 4 kt-pairs
CORR = (0, 1)        # corrected pairs (keys 0:512)
RAW = (2, 3)         # raw pairs (keys 512:1024)
SCALE = 1.0 / np.sqrt(D)
C_SHIFT = 2.5        # exp(s - C): max exp ~134 < 240 (fp8e4-safe range)
M_SCALE = 2.0 ** 13  # hs scaling into fp8e4 normal range

WAVE = True          # e-tile 0 in fp16
RAW_DIRECT = False   # raw half exp/hs write fp8 directly

_cached = {}


def build_bass(wave=WAVE, raw_direct=RAW_DIRECT):
    f16, f32 = mybir.dt.float16, mybir.dt.float32
    e4 = mybir.dt.float8e4
    DR = mybir.MatmulPerfMode.DoubleRow
    EXP = mybir.ActivationFunctionType.Exp
    nc = bacc.Bacc("TRN2", target_bir_lowering=False, debug=False, num_devices=8)

    hT = nc.dram_tensor("hT", [E, S], f16, kind="ExternalInput").ap()
    hk = nc.dram_tensor("hk", [KH, E], f16, kind="ExternalInput").ap()
    wqk = nc.dram_tensor("wqk", [P, ET * P], f16, kind="ExternalInput").ap()
    bqk = nc.dram_tensor("bqk", [P, 1], f32, kind="ExternalInput").ap()
    outT = nc.dram_tensor("outT", [E, S], f16, kind="ExternalOutput").ap()

    hT3 = hT.rearrange("(t p) q -> t p q", p=P)      # [8, 128, 2048]
    hk3 = hk.rearrange("(t p) e -> t p e", p=P)      # [8, 128, 1024]
    outT3 = outT.rearrange("(t p) q -> t p q", p=P)  # [8, 128, 2048]

    with tile.TileContext(nc) as tc:
        with (
            tc.tile_pool(name="p_in", bufs=ET) as p_in,
            tc.tile_pool(name="p_w", bufs=1) as p_w,
            tc.tile_pool(name="p_soft", bufs=KT) as p_soft,
            tc.tile_pool(name="p_f8", bufs=NP) as p_f8,
            tc.tile_pool(name="p_out", bufs=ET) as p_out,
        ):
            # ---- input DMAs ----
            # hT streams in q-column halves: sweep 1 (q 0:1024) feeds the
            # P1 projection so the exp pipeline starts after HALF the input
            # stream; sweep 2 (q 1024:2048) feeds P2 in the exp shadow.
            wqk_sb = p_w.tile([P, ET, P], f16, tag="wqk")
            nc.sync.dma_start(wqk_sb[:, 0, :], wqk[:, 0:P])
            hTa = [p_in.tile([P, S // 2], f16, tag="hTa", name=f"hTa_{et}")
                   for et in range(ET)]
            hTb = [p_in.tile([P, S // 2], f16, tag="hTb", name=f"hTb_{et}")
                   for et in range(ET)]
            nc.sync.dma_start(hTa[0][:], hT3[0][:, 0:1024])
            nc.sync.dma_start(
                wqk_sb[:, 1:, :].rearrange("p t c -> p (t c)"),
                wqk[:, P:ET * P])
            for et in range(1, ET):
                nc.sync.dma_start(hTa[et][:], hT3[et][:, 0:1024])
            bqk_sb = p_w.tile([P, 1], f32, tag="bqk")
            nc.sync.dma_start(bqk_sb[:], bqk[:])
            for et in range(ET):
                nc.sync.dma_start(hTb[et][:], hT3[et][:, 1024:2048])
            hk_sb = []
            for kt in range(KT):
                t = p_in.tile([P, E], f16, tag="hk", name=f"hk_{kt}")
                nc.sync.dma_start(t[:], hk3[kt])
                hk_sb.append(t)

            QT16h = [p_w.tile([D, S // 2], f16, tag=f"qt{h}", name=f"QT16_{h}")
                     for h in range(2)]
            KT16 = p_w.tile([D, KH], f16, tag="kt16")
            cshift = p_w.tile([P, 1], f32, tag="cshift")
            nc.gpsimd.memset(cshift[:], -C_SHIFT)
            # dummy activation: pulls the 1.3us activation-table load into
            # the DMA-bound startup window instead of the exp critical path
            warm = p_w.tile([P, 1], f32, tag="warm")
            nc.scalar.activation(warm[:], cshift[:], EXP)

            # persistent operand tiles
            w16 = []      # per kt: [128, 2048] f16
            hs16 = []     # per kt: [128, 1024] f16
            ssum_a = p_w.tile([P, KT], f32, tag="ssum_a")
            ssum_b = p_w.tile([P, KT], f32, tag="ssum_b")
            rinv = p_w.tile([P, KT], f32, tag="rinv")
            Wp = [p_f8.tile([P, 2, S], e4, tag="W8", name=f"W8_{p}")
                  for p in range(NP)]
            vp = {p: p_f8.tile([P, 2, S], e4, tag="v8", name=f"v8_{p}")
                  for p in CORR}
            Hp = [p_f8.tile([P, 2, E], e4, tag="H8", name=f"H8_{p}")
                  for p in range(NP)]
            hp = {p: p_f8.tile([P, 2, E], e4, tag="h8", name=f"h8_{p}")
                  for p in CORR}

            ot_et = {}

            def evict(et, i, eng):
                if et not in ot_et:
                    ot_et[et] = p_out.tile([P, S], f16, tag="ot",
                                           name=f"ot_{et}")
                eng(ot_et[et][:, i * 512:(i + 1) * 512], psc[(et, i)][:])

            def dma_q(et, i):
                nc.sync.dma_start(
                    outT3[et][:, i * 512:(i + 1) * 512],
                    ot_et[et][:, i * 512:(i + 1) * 512])

            psc = {}

            # ---- phase A (P1) + sweep h0 ----
            with tc.tile_pool(name="ps_p2", bufs=1, space="PSUM") as ps_p2:
                P2 = ps_p2.tile([D, S // 2], f32, tag="p2")
                with tc.tile_pool(name="ps_p1", bufs=1,
                                  space="PSUM") as ps_p1:
                    P1 = ps_p1.tile([P, S // 2], f32, tag="p1")
                    for et in range(ET):
                        st, sp = (et == 0), (et == ET - 1)
                        for c in range(2):
                            nc.tensor.matmul(
                                P1[:, c * 512:(c + 1) * 512],
                                wqk_sb[:, et, :],
                                hTa[et][:, c * 512:(c + 1) * 512],
                                start=st, stop=sp)
                    # KT bias halves on ACT (idle until the exps), QT0 bias
                    # halves on DVE, in parallel, so kt0's scores start fast
                    # two P1 readers only (same-tile readers serialize
                    # ~220ns apart): Q bias on ACT (1038 < DVE's 1192),
                    # K bias on DVE
                    with tc.high_priority():
                        nc.scalar.activation(
                            QT16h[0][:], P1[0:D, :],
                            mybir.ActivationFunctionType.Identity,
                            bias=bqk_sb[0:D, 0:1])
                        nc.vector.tensor_scalar_add(
                            KT16[:], P1[D:P, :], bqk_sb[D:P, 0:1])

                with tc.tile_pool(name="ps_b0", bufs=3,
                                  space="PSUM") as ps_b0:
                    # -- sweep h0: scores+exp on q 0:1024; P2 fills PE --
                    for kt in range(KT):
                        pi, j = divmod(kt, 2)
                        wt = p_soft.tile([P, S], f16, tag="w16",
                                         name=f"w16_{kt}")
                        w16.append(wt)
                        sc = ps_b0.tile([P, S // 2], f32, tag="sc",
                                        name=f"sc_{kt}_0")
                        for c in range(2):
                            nc.tensor.matmul(
                                sc[:, c * 512:(c + 1) * 512],
                                KT16[:, kt * P:(kt + 1) * P],
                                QT16h[0][:, c * 512:(c + 1) * 512],
                                start=True, stop=True)
                        nc.scalar.activation(
                            wt[:, 0:1024], sc[:], EXP, bias=cshift[:],
                            accum_out=ssum_a[:, kt:kt + 1])
                        # P2 projection consumes sweep 2 as it lands, one
                        # e-tile ahead so its stop-matmul (and the QT1 bias
                        # behind it) clears before the h0 scores finish
                        p2_ets = ([0, 1] if kt == 0 else
                                  [kt + 1] if kt < ET - 1 else [])
                        with tc.high_priority(offset=-400):
                            for et in p2_ets:
                                for c in range(2):
                                    nc.tensor.matmul(
                                        P2[:, c * 512:(c + 1) * 512],
                                        wqk_sb[:, et, 0:D],
                                        hTb[et][:, c * 512:(c + 1) * 512],
                                        start=(et == 0), stop=(et == ET - 1))
                        if pi in CORR and kt < 2:
                            # corrected h0 conversions (DVE/Pool idle now),
                            # sorted below the sweep so they never delay the
                            # scores/exp/bias stream; kt2/3's wait until
                            # after the QT1 bias
                            with tc.high_priority(offset=-400):
                                nc.vector.tensor_copy(
                                    Wp[pi][:, j, 0:1024], wt[:, 0:1024])
                                nc.gpsimd.tensor_sub(
                                    vp[pi][:, j, 0:1024], wt[:, 0:1024],
                                    Wp[pi][:, j, 0:1024])
                    nc.vector.tensor_scalar_add(
                        QT16h[1][:], P2[0:D, :], bqk_sb[0:D, 0:1])
                    for kt in (2, 3):
                        pi, j = divmod(kt, 2)
                        nc.vector.tensor_copy(
                            Wp[pi][:, j, 0:1024], w16[kt][:, 0:1024])
                        nc.gpsimd.tensor_sub(
                            vp[pi][:, j, 0:1024], w16[kt][:, 0:1024],
                            Wp[pi][:, j, 0:1024])

            # ---- sweep h1 + phase C ----
            with tc.tile_pool(name="ps_w", bufs=4, space="PSUM") as ps_w:
                wave_psc = None
                if wave:
                    wave_psc = [ps_w.tile([P, 512], f32, tag="ops",
                                          name=f"wv_{i}") for i in range(4)]
                    for i in range(4):
                        psc[(0, i)] = wave_psc[i]

                def wave_adv(kt):
                    for i in range(4):
                        nc.tensor.matmul(
                            wave_psc[i][:], hs16[kt][:, 0:P],
                            w16[kt][:, i * 512:(i + 1) * 512],
                            start=(kt == 0), stop=(kt == KT - 1))

                conv_q = []
                with tc.tile_pool(name="ps_b1", bufs=2,
                                  space="PSUM") as ps_b1:
                    # -- sweep h1: scores+exp on q 1024:2048 + chains --
                    for kt in range(KT):
                        pi, j = divmod(kt, 2)
                        wt = w16[kt]
                        sc = ps_b1.tile([P, S // 2], f32, tag="sc",
                                        name=f"sc_{kt}_1")
                        for c in range(2):
                            nc.tensor.matmul(
                                sc[:, c * 512:(c + 1) * 512],
                                KT16[:, kt * P:(kt + 1) * P],
                                QT16h[1][:, c * 512:(c + 1) * 512],
                                start=True, stop=True)
                        nc.scalar.activation(
                            wt[:, 1024:2048], sc[:], EXP, bias=cshift[:],
                            accum_out=ssum_b[:, kt:kt + 1])
                        with tc.high_priority():
                            # ssum->rinv->hs gates the wave and H8
                            nc.vector.tensor_add(
                                rinv[:, kt:kt + 1],
                                ssum_a[:, kt:kt + 1], ssum_b[:, kt:kt + 1])
                            nc.vector.reciprocal_approx_fast(
                                rinv[:, kt:kt + 1], rinv[:, kt:kt + 1])
                            nc.vector.tensor_scalar_mul(
                                rinv[:, kt:kt + 1], rinv[:, kt:kt + 1],
                                M_SCALE)
                            hs = p_soft.tile([P, E], f16, tag="hs",
                                             name=f"hs_{kt}")
                            nc.vector.tensor_scalar_mul(
                                hs[:], hk_sb[kt][:], rinv[:, kt:kt + 1])
                        hs16.append(hs)
                        if pi in CORR:
                            # corrected h1 conversions; DVE copies deferred
                            # one kt so the next rinv->hs chain isn't stuck
                            # behind them on the in-order DVE; residual subs
                            # split across Pool/DVE by parity
                            def conv_h1(kt=kt, pi=pi, j=j, wt=wt, hs=hs):
                                nc.vector.tensor_copy(
                                    Wp[pi][:, j, 1024:2048], wt[:, 1024:2048])
                                nc.vector.tensor_copy(Hp[pi][:, j, :], hs[:])
                                if kt % 2 == 0:
                                    nc.gpsimd.tensor_sub(
                                        vp[pi][:, j, 1024:2048],
                                        wt[:, 1024:2048],
                                        Wp[pi][:, j, 1024:2048])
                                    nc.gpsimd.tensor_sub(
                                        hp[pi][:, j, :], hs[:],
                                        Hp[pi][:, j, :])
                                else:
                                    nc.vector.tensor_sub(
                                        vp[pi][:, j, 1024:2048],
                                        wt[:, 1024:2048],
                                        Wp[pi][:, j, 1024:2048])
                                    nc.vector.tensor_sub(
                                        hp[pi][:, j, :], hs[:],
                                        Hp[pi][:, j, :])
                            conv_q.append(conv_h1)
                        if conv_q and kt >= 1:
                            conv_q.pop(0)()
                        if wave and kt >= 2:
                            # sort the wave BELOW the sweep's scores/exps so
                            # the compile-time scheduler never slots a wave
                            # matmul where it delays the next scores -> exp
                            with tc.high_priority(offset=-400):
                                wave_adv(kt - 2)

                for fn in conv_q:
                    fn()
                conv_q.clear()

                # ---- phase C ----
                with tc.tile_pool(name="ps_c", bufs=4, space="PSUM") as ps_c:
                    if wave:
                        # finish + evict the wave FIRST (all on ACT, ahead of
                        # its raw-conversion queue): frees the wave's PSUM
                        # ring for group 2's corrected DRs
                        wave_adv(KT - 2)
                        wave_adv(KT - 1)
                        with tc.high_priority():
                            for i in range(4):
                                evict(0, i, nc.scalar.copy)
                                dma_q(0, i)
                        groups = ([(e, (0, 1, 2, 3))
                                   for e in range(1, ET - 2)]
                                  + [(ET - 2, (0, 1)), (ET - 2, (2, 3)),
                                     (ET - 1, (0, 1)), (ET - 1, (2,)),
                                     (ET - 1, (3,))])
                    else:
                        groups = ([(e, (0, 1, 2, 3)) for e in range(ET - 1)]
                                  + [(ET - 1, (0, 1)), (ET - 1, (2, 3))])

                    # raw-half conversions (ACT idle post-exps; DVE/Pool
                    # pick up after their corrected-pair backlog)
                    engs = {
                        4: (nc.scalar.copy, nc.scalar.copy),
                        5: (nc.gpsimd.tensor_copy, nc.gpsimd.tensor_copy),
                        6: (nc.scalar.copy, nc.gpsimd.tensor_copy),
                        7: (nc.vector.tensor_copy, nc.vector.tensor_copy),
                    }
                    for kt in (4, 5, 6, 7):
                        pi, j = divmod(kt, 2)
                        weng, heng = engs[kt]
                        weng(Wp[pi][:, j, :], w16[kt][:])
                        heng(Hp[pi][:, j, :], hs16[kt][:])
                    pools = [ps_c, ps_w]

                    def group_corr(g, pairs=CORR):
                        et, ilist = groups[g]
                        if pairs[0] == CORR[0]:
                            pool = pools[g % 2]
                            for i in ilist:
                                psc[(et, i)] = pool.tile(
                                    [P, 512], f32, tag="ops",
                                    name=f"psc_{et}_{i}")
                        for p in pairs:
                            for i in ilist:
                                es = slice(et * P, (et + 1) * P)
                                qs = slice(i * 512, (i + 1) * 512)
                                nc.tensor.matmul(
                                    psc[(et, i)][:], Hp[p][:, :, es],
                                    Wp[p][:, :, qs],
                                    start=(p == CORR[0]), stop=False,
                                    perf_mode=DR)
                                nc.tensor.matmul(
                                    psc[(et, i)][:], hp[p][:, :, es],
                                    Wp[p][:, :, qs],
                                    start=False, stop=False, perf_mode=DR)
                                nc.tensor.matmul(
                                    psc[(et, i)][:], Hp[p][:, :, es],
                                    vp[p][:, :, qs],
                                    start=False, stop=False, perf_mode=DR)

                    def group_raw_finish(g, last):
                        # raw DRs, then evict right behind each tile's final
                        # DR. The last two groups go per-tile-major with
                        # quarter DMAs so the output stream drains during
                        # the remaining DRs instead of after them.
                        et, ilist = groups[g]
                        es = slice(et * P, (et + 1) * P)
                        for i in ilist:
                            qs = slice(i * 512, (i + 1) * 512)
                            nc.tensor.matmul(
                                psc[(et, i)][:], Hp[RAW[0]][:, :, es],
                                Wp[RAW[0]][:, :, qs],
                                start=False, stop=False, perf_mode=DR)
        # early groups' evicts/DMAs have slack until their PSUM
                        # ring slot is re-needed; sort them below the raw
                        # conversions competing for DVE/ACT at C-start
                        early = g < 3
                        for i in ilist:
                            qs = slice(i * 512, (i + 1) * 512)
                            nc.tensor.matmul(
                                psc[(et, i)][:], Hp[RAW[1]][:, :, es],
                                Wp[RAW[1]][:, :, qs],
                                start=False, stop=True, perf_mode=DR)
                            with tc.high_priority(offset=-150 if early else None):
                                evict(et, i, (nc.vector.tensor_copy
                                              if i % 2 == 0
                                              else nc.scalar.copy))
                            if g == len(groups) - 2 and i >= 2:
                                # penultimate tile's quarter via the Pool
                                # SWDGE path: no HWDGE slot, so the final
                                # quarter issues without queueing
                                nc.gpsimd.dma_start(
                                    outT3[et][:, i * 512:(i + 1) * 512],
                                    ot_et[et][:, i * 512:(i + 1) * 512])
                            elif g == len(groups) - 1 and i >= 2:
                                dma_q(et, i)
                            elif i == 1:
                                nc.sync.dma_start(outT3[et][:, 0:1024],
                                                  ot_et[et][:, 0:1024])
                            elif i == 3:
                                nc.sync.dma_start(outT3[et][:, 1024:2048],
                                                  ot_et[et][:, 1024:2048])

                    ng = len(groups)
                    # first two groups pair-interleaved: G1-p0 fills the PE
                    # while kt2/3's conversions finish for the p1 DRs
                    group_corr(0, (CORR[0],))
                    group_corr(1, (CORR[0],))
                    group_corr(0, (CORR[1],))
                    group_corr(1, (CORR[1],))
                    for g in range(ng):
                        if 1 <= g and g + 1 < ng:
                            group_corr(g + 1)
                        group_raw_finish(g, last=(g == ng - 1))

    nc.compile()
    return nc


def _prep_in_maps(h, Wq, bq, Wk, bk):
    wq16 = (np.asarray(Wq, np.float32) * SCALE).astype(np.float16)
    wk16 = np.asarray(Wk, np.float32).astype(np.float16)
    # per e-tile stationary [128, 128] = [Wq' | Wk] rows for that tile
    wqk = np.concatenate(
        [wq16.reshape(ET, P, D), wk16.reshape(ET, P, D)], axis=2)  # [ET,128,128]
    wqk = np.ascontiguousarray(wqk.transpose(1, 0, 2).reshape(P, ET * P))
    bqk = np.ascontiguousarray(np.concatenate(
        [np.asarray(bq, np.float32) * SCALE,
         np.asarray(bk, np.float32)]).reshape(P, 1))
    in_maps = []
    for c in range(8):
        b, half = divmod(c, 2)
        hb = np.asarray(h[b], np.float32)
        rolled = np.roll(hb, -KH * half, axis=0) if half else hb
        h16 = rolled.astype(np.float16)
        in_maps.append({
            "hT": np.ascontiguousarray(h16.T),
            "hk": np.ascontiguousarray(h16[0:KH]),
            "wqk": wqk, "bqk": bqk,
        })
    return in_maps


def _assemble(results):
    out = np.empty((B, S, E), np.float32)
    inv_m = np.float32(1.0 / M_SCALE)
    for b in range(B):
        p0 = results[2 * b]["outT"].astype(np.float32).T
        p1 = results[2 * b + 1]["outT"].astype(np.float32).T
        out[b] = (p0 + np.roll(p1, KH, axis=0)) * inv_m
    return out


def kernel(h, Wq, bq, Wk, bk, Wv=None, bv=None, **_unused):
    if "nc" not in _cached:
        _cached["nc"] = build_bass()
    nc = _cached["nc"]
    in_maps = _prep_in_maps(h, Wq, bq, Wk, bk)
    res = run_bass_kernel_spmd(nc, in_maps, list(range(8)))
    return _assemble(res.results)
